# revision 8
# baseline (speedup 1.0000x reference)
"""Trainium2 Bass kernel for nn_FMG_6717328851807 (dense_transformer).

Reference computation (B=8, C=512, H=W=64, K=64, MEM=512, heads=8, d=64):
    q = Wq @ x            (1x1 conv)          -> [B,h,N,d], N = H*W = 4096
    k = Ft @ Wk.T, v = Ft @ Wv.T              -> [B,h,K,d]
    attn = softmax(q k^T / sqrt(d))           -> [B,h,N,K]
    out = attn @ v                            -> [B,h,N,d]
    y = x + Wp @ out + bp

Sharding: pure data-parallel over B — one batch element per NeuronCore,
no collectives. Within a core everything is computed in "transposed"
layout (channels on partitions, spatial N on the free dim) so every
matmul runs with a 512-wide moving operand in float32r (full PE rate,
fp32 storage, no dtype casts):

    qT[C,N]      = WqT.T @ x            (16 MMs / 512-col chunk)
    kT[C,K]      = WkT.T @ FtT          (once)
    v[K,C]       = FtT.T @ WvT          (once, duplicated on partitions
                                         0-63 and 64-127 for pair-packing)
    scoresT[k,n] = kT_h.T @ qT_h        (heads packed in pairs onto the
                                         128 partitions; the even/odd head
                                         matmuls target disjoint 64-row/col
                                         groups of the PE array and run
                                         concurrently)
    expT         = exp(scoresT / 8)     (ScalarE, PSUM -> SBUF)
    sums[8,n]    = blockones.T @ expT   (PE, accumulated over head pairs)
    outT_h       = v_h.T @ expT_h       (pair-packed like scoresT)
    outT        *= 1/sums               (partition-broadcast via a DRAM
                                         bounce + 0-step DMA read, DVE mul)
    y            = WpT.T @ outT + bp + x
"""

import numpy as np

import concourse.bass as bass
import concourse.mybir as mybir
import concourse.tile as tile
from concourse import bacc
from concourse.bass_utils import run_bass_kernel_spmd

F32 = mybir.dt.float32
F32R = mybir.dt.float32r
BF16 = mybir.dt.bfloat16

B, C, N = 8, 512, 4096
HW = 64
K, MEM, H, D = 64, 512, 8, 64
NW = 512                # columns of N processed per chunk
NCH = N // NW           # 8 chunks
CCH = C // 128          # 4 chunks of channels/partitions
N_CORES = 8


def _pbcast_src(row_ap):
    """AP reading one DRAM row, broadcast over 64 partitions."""
    ap = [[0, 64]] + [list(p) for p in row_ap.ap[1:]]
    return bass.AP(tensor=row_ap.tensor, offset=row_ap.offset, ap=ap)


def build_bass():
    nc = bacc.Bacc("TRN2", target_bir_lowering=False, debug=False)

    xb = nc.dram_tensor("xb", [C, N], F32, kind="ExternalInput")
    ftT = nc.dram_tensor("ftT", [MEM, K], F32, kind="ExternalInput")
    wqT = nc.dram_tensor("wqT", [C, C], F32, kind="ExternalInput")
    wkT = nc.dram_tensor("wkT", [MEM, C], F32, kind="ExternalInput")
    wvT = nc.dram_tensor("wvT", [MEM, C], F32, kind="ExternalInput")
    wpT = nc.dram_tensor("wpT", [C, C], F32, kind="ExternalInput")
    bpv = nc.dram_tensor("bpv", [C, 1], F32, kind="ExternalInput")
    onesb = nc.dram_tensor("onesb", [C, H], BF16, kind="ExternalInput")
    yb = nc.dram_tensor("yb", [C, N], F32, kind="ExternalOutput")

    with tile.TileContext(nc) as tc:
        _body(tc, xb, ftT, wqT, wkT, wvT, wpT, bpv, onesb, yb)
    nc.compile()
    return nc


def _body(tc, xb, ftT, wqT, wkT, wvT, wpT, bpv, onesb, yb):
    nc = tc.nc
    Exp = mybir.ActivationFunctionType.Exp
    Ident = mybir.ActivationFunctionType.Identity

    with (
        tc.tile_pool(name="const", bufs=1) as const,
        tc.tile_pool(name="xin", bufs=3) as xin,
        tc.tile_pool(name="qt", bufs=2) as qtp,
        tc.tile_pool(name="expt", bufs=2) as expp,
        tc.tile_pool(name="bcast", bufs=2) as bcp,
        tc.tile_pool(name="outt", bufs=2) as outp,
        tc.tile_pool(name="ytmp", bufs=2) as ytp,
        tc.tile_pool(name="yout", bufs=2) as yop,
        tc.tile_pool(name="recip", bufs=2) as rcp,
        tc.tile_pool(name="rdram", bufs=2, space="DRAM") as rdp,
        tc.tile_pool(name="ps_qy", bufs=3, space="PSUM") as ps_qy,
        tc.tile_pool(name="ps_s", bufs=2, space="PSUM") as ps_s,
        tc.tile_pool(name="ps_sum", bufs=1, space="PSUM") as ps_sum,
        tc.tile_pool(name="ps_o", bufs=2, space="PSUM") as ps_o,
    ):
        # ---- load constants (f32r: same bits, rounded inside the PE) -------
        def load_rows(dram, ncols, dtype=F32R):
            tiles = []
            for j in range(CCH):
                t = const.tile([128, ncols], dtype, tag=f"{dram.name}{j}")
                nc.sync.dma_start(
                    out=t[:], in_=dram[128 * j:128 * (j + 1), :].bitcast(dtype)
                )
                tiles.append(t)
            return tiles

        wq_sb = load_rows(wqT, C)
        wp_sb = load_rows(wpT, C)
        wk_sb = load_rows(wkT, C)
        wv_sb = load_rows(wvT, C)
        ftT_sb = load_rows(ftT, K)
        ones_sb = load_rows(onesb, H, BF16)
        bp_sb = const.tile([128, CCH], F32, tag="bp")
        nc.sync.dma_start(
            out=bp_sb[:], in_=bpv.rearrange("(m p) one -> p (m one)", p=128)
        )

        # ---- kT = Wk @ Ft^T  [C, K] ----------------------------------------
        kT_sb = []
        for cj in range(CCH):
            pk = ps_s.tile([128, NW], F32, tag="ps")
            for mk in range(CCH):
                nc.tensor.matmul(
                    pk[:, :K],
                    lhsT=wk_sb[mk][:, 128 * cj:128 * (cj + 1)],
                    rhs=ftT_sb[mk][:],
                    start=(mk == 0),
                    stop=(mk == CCH - 1),
                )
            t = const.tile([128, K], BF16, tag=f"kT{cj}")
            nc.scalar.copy(t[:], pk[:, :K])
            kT_sb.append(t)

        # ---- v = Ft @ Wv^T  [K, C], duplicated on both partition halves ----
        v_dup = const.tile([128, C], BF16, tag="vdup")
        pv = ps_o.tile([128, NW], F32, tag="po")
        for mk in range(CCH):
            nc.tensor.matmul(
                pv[0:64, :],
                lhsT=ftT_sb[mk][:],
                rhs=wv_sb[mk][:],
                start=(mk == 0),
                stop=(mk == CCH - 1),
            )
        nc.scalar.copy(v_dup[0:64, :], pv[0:64, :])
        nc.sync.dma_start(out=v_dup[64:128, :], in_=v_dup[0:64, :])

        # ---- main loop over spatial chunks ---------------------------------
        for i in range(NCH):
            csl = bass.ts(i, NW)

            x_sb = []
            for j in range(CCH):
                t = xin.tile([128, NW], F32R, tag=f"x{j}")
                nc.sync.dma_start(
                    out=t[:], in_=xb[128 * j:128 * (j + 1), csl].bitcast(F32R)
                )
                x_sb.append(t)

            # qT chunk [C, NW]
            qT_sb = []
            for m in range(CCH):
                pq = ps_qy.tile([128, NW], F32, tag="qy")
                for k2 in range(CCH):
                    nc.tensor.matmul(
                        pq[:],
                        lhsT=wq_sb[k2][:, 128 * m:128 * (m + 1)],
                        rhs=x_sb[k2][:],
                        start=(k2 == 0),
                        stop=(k2 == CCH - 1),
                    )
                t = qtp.tile([128, NW], BF16, tag=f"q{m}")
                nc.scalar.copy(t[:], pq[:])
                qT_sb.append(t)

            # scoresT + exp, head pair j = heads (2j, 2j+1)
            expT_sb = []
            for j in range(CCH):
                ps = ps_s.tile([128, NW], F32, tag="ps")
                for half in range(2):
                    lo, hi = 64 * half, 64 * half + 64
                    nc.tensor.matmul(
                        ps[lo:hi, :],
                        lhsT=kT_sb[j][lo:hi, :],
                        rhs=qT_sb[j][lo:hi, :],
                        start=True,
                        stop=True,
                    )
                t = expp.tile([128, NW], BF16, tag=f"e{j}")
                nc.scalar.activation(t[:], ps[:], Exp, bias=0.0, scale=0.125)
                expT_sb.append(t)

            # per-head softmax denominators [H, NW], then reciprocal
            psum = ps_sum.tile([8, NW], F32, tag="psum")
            for j in range(CCH):
                nc.tensor.matmul(
                    psum[:],
                    lhsT=ones_sb[j][:, :8],
                    rhs=expT_sb[j][:],
                    start=(j == 0),
                    stop=(j == CCH - 1),
                )
            recip = rcp.tile([8, NW], F32, tag="recip")
            nc.vector.reciprocal(recip[:], psum[:])
            recip_d = rdp.tile([8, NW], F32, tag="recip_d")
            nc.sync.dma_start(out=recip_d[:], in_=recip[:])

            # broadcast 1/sum rows across the partition halves
            bc_sb = []
            for j in range(CCH):
                t = bcp.tile([128, NW], F32, tag=f"b{j}")
                nc.sync.dma_start(
                    out=t[0:64, :], in_=_pbcast_src(recip_d[2 * j:2 * j + 1, :])
                )
                nc.sync.dma_start(
                    out=t[64:128, :],
                    in_=_pbcast_src(recip_d[2 * j + 1:2 * j + 2, :]),
                )
                bc_sb.append(t)

            # outT = v^T @ expT per head (pair-packed), then normalize
            outT_sb = []
            for j in range(CCH):
                po = ps_o.tile([128, NW], F32, tag="po")
                for half in range(2):
                    lo, hi = 64 * half, 64 * half + 64
                    nc.tensor.matmul(
                        po[lo:hi, :],
                        lhsT=v_dup[lo:hi, 128 * j + 64 * half:
                                   128 * j + 64 * half + 64],
                        rhs=expT_sb[j][lo:hi, :],
                        start=True,
                        stop=True,
                    )
                t = outp.tile([128, NW], F32R, tag=f"o{j}")
                nc.vector.tensor_mul(t[:], po[:], bc_sb[j][:])
                outT_sb.append(t)

            # y = Wp @ outT + bp + x
            for m in range(CCH):
                py = ps_qy.tile([128, NW], F32, tag="qy")
                for k2 in range(CCH):
                    nc.tensor.matmul(
                        py[:],
                        lhsT=wp_sb[k2][:, 128 * m:128 * (m + 1)],
                        rhs=outT_sb[k2][:],
                        start=(k2 == 0),
                        stop=(k2 == CCH - 1),
                    )
                yt = ytp.tile([128, NW], F32, tag="yt")
                nc.scalar.activation(
                    yt[:], py[:], Ident, bias=bp_sb[:, m:m + 1], scale=1.0
                )
                yo = yop.tile([128, NW], F32, tag=f"yo{m}")
                nc.vector.tensor_add(yo[:], yt[:], x_sb[m][:].bitcast(F32))
                nc.sync.dma_start(out=yb[128 * m:128 * (m + 1), csl], in_=yo[:])


_NC_CACHE = None
LAST_RESULTS = None


def kernel(x, Ft, Wq, Wk, Wv, Wp, bp):
    global _NC_CACHE, LAST_RESULTS
    x = np.ascontiguousarray(np.asarray(x, dtype=np.float32))
    Ft = np.asarray(Ft, dtype=np.float32)

    wqT = np.ascontiguousarray(np.asarray(Wq, dtype=np.float32).T)
    wkT = np.ascontiguousarray(np.asarray(Wk, dtype=np.float32).T)
    wvT = np.ascontiguousarray(np.asarray(Wv, dtype=np.float32).T)
    wpT = np.ascontiguousarray(np.asarray(Wp, dtype=np.float32).T)
    bpv = np.ascontiguousarray(np.asarray(bp, dtype=np.float32).reshape(C, 1))
    import ml_dtypes
    onesb = np.zeros((C, H), dtype=np.float32)
    onesb[np.arange(C), np.arange(C) // D] = 1.0
    onesb = onesb.astype(ml_dtypes.bfloat16)
    ftT = np.ascontiguousarray(Ft.transpose(0, 2, 1))
    xr = x.reshape(B, C, N)

    if _NC_CACHE is None:
        _NC_CACHE = build_bass()
    nc = _NC_CACHE

    in_maps = [
        {
            "xb": xr[b],
            "ftT": ftT[b],
            "wqT": wqT,
            "wkT": wkT,
            "wvT": wvT,
            "wpT": wpT,
            "bpv": bpv,
            "onesb": onesb,
        }
        for b in range(B)
    ]
    res = run_bass_kernel_spmd(nc, in_maps, core_ids=list(range(N_CORES)))
    LAST_RESULTS = res
    y = np.stack([res.results[b]["yb"] for b in range(B)])
    return y.reshape(B, C, HW, HW)


# revision 12
# speedup vs baseline: 1.2603x; 1.2603x over previous
"""Trainium2 Bass kernel for nn_FMG_6717328851807 (dense_transformer).

Reference computation (B=8, C=512, H=W=64, K=64, MEM=512, heads=8, d=64):
    q = Wq @ x            (1x1 conv)          -> [B,h,N,d], N = H*W = 4096
    k = Ft @ Wk.T, v = Ft @ Wv.T              -> [B,h,K,d]
    attn = softmax(q k^T / sqrt(d))           -> [B,h,N,K]
    out = attn @ v                            -> [B,h,N,d]
    y = x + Wp @ out + bp

Sharding: pure data-parallel over B — one batch element per NeuronCore,
no collectives. Within a core everything is computed in "transposed"
layout (channels on partitions, spatial N on the free dim) so every
matmul runs with a 512-wide moving operand in float32r (full PE rate,
fp32 storage, no dtype casts):

    qT[C,N]      = WqT.T @ x            (16 MMs / 512-col chunk)
    kT[C,K]      = WkT.T @ FtT          (once)
    v[K,C]       = FtT.T @ WvT          (once, duplicated on partitions
                                         0-63 and 64-127 for pair-packing)
    scoresT[k,n] = kT_h.T @ qT_h        (heads packed in pairs onto the
                                         128 partitions; the even/odd head
                                         matmuls target disjoint 64-row/col
                                         groups of the PE array and run
                                         concurrently)
    expT         = exp(scoresT / 8)     (ScalarE, PSUM -> SBUF)
    sums[8,n]    = blockones.T @ expT   (PE, accumulated over head pairs)
    outT_h       = v_h.T @ expT_h       (pair-packed like scoresT)
    outT        *= 1/sums               (partition-broadcast via a DRAM
                                         bounce + 0-step DMA read, DVE mul)
    y            = WpT.T @ outT + bp + x
"""

import numpy as np

import concourse.bass as bass
import concourse.mybir as mybir
import concourse.tile as tile
from concourse import bacc
from concourse.bass_utils import run_bass_kernel_spmd

F32 = mybir.dt.float32
F32R = mybir.dt.float32r
BF16 = mybir.dt.bfloat16

B, C, N = 8, 512, 4096
HW = 64
K, MEM, H, D = 64, 512, 8, 64
NW = 512                # columns of N processed per chunk
NCH = N // NW           # 8 chunks
CCH = C // 128          # 4 chunks of channels/partitions
N_CORES = 8


def _pbcast_src(row_ap):
    """AP reading one DRAM row, broadcast over 64 partitions."""
    ap = [[0, 64]] + [list(p) for p in row_ap.ap[1:]]
    return bass.AP(tensor=row_ap.tensor, offset=row_ap.offset, ap=ap)


def build_bass():
    nc = bacc.Bacc("TRN2", target_bir_lowering=False, debug=False)

    xb = nc.dram_tensor("xb", [C, N], F32, kind="ExternalInput")
    ftT = nc.dram_tensor("ftT", [MEM, K], F32, kind="ExternalInput")
    wqT = nc.dram_tensor("wqT", [C, C], F32, kind="ExternalInput")
    wkT = nc.dram_tensor("wkT", [MEM, C], F32, kind="ExternalInput")
    wvT = nc.dram_tensor("wvT", [MEM, C], F32, kind="ExternalInput")
    wpT = nc.dram_tensor("wpT", [C, C], F32, kind="ExternalInput")
    bpv = nc.dram_tensor("bpv", [C, 1], F32, kind="ExternalInput")
    onesb = nc.dram_tensor("onesb", [C, H], BF16, kind="ExternalInput")
    yb = nc.dram_tensor("yb", [C, N], F32, kind="ExternalOutput")

    with tile.TileContext(nc) as tc:
        _body(tc, xb, ftT, wqT, wkT, wvT, wpT, bpv, onesb, yb)
    nc.compile()
    return nc


def _body(tc, xb, ftT, wqT, wkT, wvT, wpT, bpv, onesb, yb):
    nc = tc.nc
    Exp = mybir.ActivationFunctionType.Exp
    Ident = mybir.ActivationFunctionType.Identity

    with (
        tc.tile_pool(name="const", bufs=1) as const,
        tc.tile_pool(name="xin", bufs=3) as xin,
        tc.tile_pool(name="qt", bufs=2) as qtp,
        tc.tile_pool(name="expt", bufs=2) as expp,
        tc.tile_pool(name="bcast", bufs=2) as bcp,
        tc.tile_pool(name="outt", bufs=2) as outp,
        tc.tile_pool(name="ytmp", bufs=2) as ytp,
        tc.tile_pool(name="yout", bufs=2) as yop,
        tc.tile_pool(name="recip", bufs=2) as rcp,
        tc.tile_pool(name="rdram", bufs=2, space="DRAM") as rdp,
        tc.tile_pool(name="ps_qy", bufs=3, space="PSUM") as ps_qy,
        tc.tile_pool(name="ps_s", bufs=2, space="PSUM") as ps_s,
        tc.tile_pool(name="ps_sum", bufs=1, space="PSUM") as ps_sum,
        tc.tile_pool(name="ps_o", bufs=2, space="PSUM") as ps_o,
    ):
        # ---- load constants (f32r: same bits, rounded inside the PE) -------
        def load_rows(dram, ncols, dtype=F32R):
            tiles = []
            for j in range(CCH):
                t = const.tile([128, ncols], dtype, tag=f"{dram.name}{j}")
                nc.sync.dma_start(
                    out=t[:], in_=dram[128 * j:128 * (j + 1), :].bitcast(dtype)
                )
                tiles.append(t)
            return tiles

        wq_sb = load_rows(wqT, C)
        wp_sb = load_rows(wpT, C)
        wk_sb = load_rows(wkT, C)
        wv_sb = load_rows(wvT, C)
        ftT_sb = load_rows(ftT, K)
        ones_sb = load_rows(onesb, H, BF16)
        bp_sb = const.tile([128, CCH], F32, tag="bp")
        nc.sync.dma_start(
            out=bp_sb[:], in_=bpv.rearrange("(m p) one -> p (m one)", p=128)
        )

        # ---- kT = Wk @ Ft^T  [C, K] ----------------------------------------
        kT_sb = []
        for cj in range(CCH):
            pk = ps_s.tile([128, NW], F32, tag="ps")
            for mk in range(CCH):
                nc.tensor.matmul(
                    pk[:, :K],
                    lhsT=wk_sb[mk][:, 128 * cj:128 * (cj + 1)],
                    rhs=ftT_sb[mk][:],
                    start=(mk == 0),
                    stop=(mk == CCH - 1),
                )
            t = const.tile([128, K], BF16, tag=f"kT{cj}")
            nc.scalar.copy(t[:], pk[:, :K])
            kT_sb.append(t)

        # ---- v = Ft @ Wv^T  [K, C], duplicated on both partition halves ----
        v_dup = const.tile([128, C], BF16, tag="vdup")
        pv = ps_o.tile([128, NW], F32, tag="po")
        for mk in range(CCH):
            nc.tensor.matmul(
                pv[0:64, :],
                lhsT=ftT_sb[mk][:],
                rhs=wv_sb[mk][:],
                start=(mk == 0),
                stop=(mk == CCH - 1),
            )
        nc.scalar.copy(v_dup[0:64, :], pv[0:64, :])
        nc.sync.dma_start(out=v_dup[64:128, :], in_=v_dup[0:64, :])

        # ---- pipelined y-projection stage (one chunk behind) ---------------
        def y_stage(i, outT_sb, x_big):
            csl = bass.ts(i, NW)
            yo = yop.tile([128, CCH, NW], F32, tag="yo")
            for m in range(CCH):
                py = ps_qy.tile([128, NW], F32, tag="qy")
                for k2 in range(CCH):
                    nc.tensor.matmul(
                        py[:],
                        lhsT=wp_sb[k2][:, 128 * m:128 * (m + 1)],
                        rhs=outT_sb[k2][:],
                        start=(k2 == 0),
                        stop=(k2 == CCH - 1),
                    )
                yt = ytp.tile([128, NW], F32, tag="yt")
                nc.scalar.activation(
                    yt[:], py[:], Ident, bias=bp_sb[:, m:m + 1], scale=1.0
                )
                nc.vector.tensor_add(
                    yo[:, m, :], yt[:], x_big[:, m, :].bitcast(F32)
                )
            nc.sync.dma_start(
                out=yb[:, csl].rearrange("(m p) c -> p m c", p=128), in_=yo[:]
            )

        # ---- main loop over spatial chunks ---------------------------------
        prev = None
        for i in range(NCH):
            csl = bass.ts(i, NW)

            x_big = xin.tile([128, CCH, NW], F32R, tag="x")
            nc.sync.dma_start(
                out=x_big[:],
                in_=xb[:, csl].rearrange("(j p) c -> p j c", p=128).bitcast(F32R),
            )

            # qT chunk [C, NW]
            qT_sb = []
            for m in range(CCH):
                pq = ps_qy.tile([128, NW], F32, tag="qy")
                for k2 in range(CCH):
                    nc.tensor.matmul(
                        pq[:],
                        lhsT=wq_sb[k2][:, 128 * m:128 * (m + 1)],
                        rhs=x_big[:, k2, :],
                        start=(k2 == 0),
                        stop=(k2 == CCH - 1),
                    )
                t = qtp.tile([128, NW], BF16, tag=f"q{m}")
                nc.scalar.copy(t[:], pq[:])
                qT_sb.append(t)

            # scoresT + exp, head pair j = heads (2j, 2j+1)
            expT_sb = []
            for j in range(CCH):
                ps = ps_s.tile([128, NW], F32, tag="ps")
                for half in range(2):
                    lo, hi = 64 * half, 64 * half + 64
                    nc.tensor.matmul(
                        ps[lo:hi, :],
                        lhsT=kT_sb[j][lo:hi, :],
                        rhs=qT_sb[j][lo:hi, :],
                        start=True,
                        stop=True,
                    )
                t = expp.tile([128, NW], BF16, tag=f"e{j}")
                nc.scalar.activation(t[:], ps[:], Exp, bias=0.0, scale=0.125)
                expT_sb.append(t)

            # per-head softmax denominators [H, NW], then reciprocal
            psum = ps_sum.tile([8, NW], F32, tag="psum")
            for j in range(CCH):
                nc.tensor.matmul(
                    psum[:],
                    lhsT=ones_sb[j][:, :8],
                    rhs=expT_sb[j][:],
                    start=(j == 0),
                    stop=(j == CCH - 1),
                )
            recip = rcp.tile([8, NW], F32, tag="recip")
            nc.vector.reciprocal(recip[:], psum[:])
            recip_d = rdp.tile([8, NW], F32, tag="recip_d")
            nc.sync.dma_start(out=recip_d[:], in_=recip[:])

            # broadcast 1/sum rows across the partition halves
            bc_sb = []
            for j in range(CCH):
                t = bcp.tile([128, NW], F32, tag=f"b{j}")
                nc.sync.dma_start(
                    out=t[0:64, :], in_=_pbcast_src(recip_d[2 * j:2 * j + 1, :])
                )
                nc.sync.dma_start(
                    out=t[64:128, :],
                    in_=_pbcast_src(recip_d[2 * j + 1:2 * j + 2, :]),
                )
                bc_sb.append(t)

            # y projection for the previous chunk (overlaps this chunk's
            # softmax-normalize latency chain)
            if prev is not None:
                y_stage(i - 1, *prev)

            # outT = v^T @ expT per head (pair-packed), then normalize
            outT_sb = []
            for j in range(CCH):
                po = ps_o.tile([128, NW], F32, tag="po")
                for half in range(2):
                    lo, hi = 64 * half, 64 * half + 64
                    nc.tensor.matmul(
                        po[lo:hi, :],
                        lhsT=v_dup[lo:hi, 128 * j + 64 * half:
                                   128 * j + 64 * half + 64],
                        rhs=expT_sb[j][lo:hi, :],
                        start=True,
                        stop=True,
                    )
                t = outp.tile([128, NW], F32R, tag=f"o{j}")
                nc.vector.tensor_mul(t[:], po[:], bc_sb[j][:])
                outT_sb.append(t)

            prev = (outT_sb, x_big)

        y_stage(NCH - 1, *prev)


_NC_CACHE = None
LAST_RESULTS = None


def kernel(x, Ft, Wq, Wk, Wv, Wp, bp):
    global _NC_CACHE, LAST_RESULTS
    x = np.ascontiguousarray(np.asarray(x, dtype=np.float32))
    Ft = np.asarray(Ft, dtype=np.float32)

    wqT = np.ascontiguousarray(np.asarray(Wq, dtype=np.float32).T)
    wkT = np.ascontiguousarray(np.asarray(Wk, dtype=np.float32).T)
    wvT = np.ascontiguousarray(np.asarray(Wv, dtype=np.float32).T)
    wpT = np.ascontiguousarray(np.asarray(Wp, dtype=np.float32).T)
    bpv = np.ascontiguousarray(np.asarray(bp, dtype=np.float32).reshape(C, 1))
    import ml_dtypes
    onesb = np.zeros((C, H), dtype=np.float32)
    onesb[np.arange(C), np.arange(C) // D] = 1.0
    onesb = onesb.astype(ml_dtypes.bfloat16)
    ftT = np.ascontiguousarray(Ft.transpose(0, 2, 1))
    xr = x.reshape(B, C, N)

    if _NC_CACHE is None:
        _NC_CACHE = build_bass()
    nc = _NC_CACHE

    in_maps = [
        {
            "xb": xr[b],
            "ftT": ftT[b],
            "wqT": wqT,
            "wkT": wkT,
            "wvT": wvT,
            "wpT": wpT,
            "bpv": bpv,
            "onesb": onesb,
        }
        for b in range(B)
    ]
    res = run_bass_kernel_spmd(nc, in_maps, core_ids=list(range(N_CORES)))
    LAST_RESULTS = res
    y = np.stack([res.results[b]["yb"] for b in range(B)])
    return y.reshape(B, C, HW, HW)
